# revision 6
# baseline (speedup 1.0000x reference)
"""Trainium2 Bass kernel for the Camera ISP pipeline (mosaic -> gaussian blur
-> per-channel piecewise-linear calibration -> noise -> Malvar demosaic -> clip).

v2 strategy (per core, pure data parallel over batch: 4 images/core):
- Row-decimated contiguous DMA loads of the 4 bayer source planes (f32r,
  full width); noise and output travel as fp16 (halves their DMA bytes).
- Vertical blur directly in the quad (polyphase) domain: banded matmuls on
  PE with free dim 256 (only the column parity each family needs).
- Horizontal blur on Pool reading the vblur PSUM directly (no evacuation).
- 17-knot np.interp evaluated exactly by chained custom DVE instructions
  (2 piecewise segments per instruction); the G quads share one chain.
  Optionally the last OFFLOAD_C kinks move to ACT (relu planes) + PE
  (diagonal-band PSUM accumulation) to debottleneck DVE.
- Malvar 5x5 demosaic as fp16 banded matmuls accumulating in PSUM.
- Clip/assembly split across ACT/Pool/DVE by knob; fp16 output tiles,
  3 output DMAs per image.
"""

import sys

sys.path.insert(0, "/opt/trn_rl_repo")

import numpy as np

import concourse.bacc as bacc
import concourse.bass as bass
import concourse.tile as tile
from concourse import mybir
from concourse.bass_utils import run_bass_kernel_spmd
from concourse import dve_ops as _dops
from concourse.dve_spec import (
    C0, C1, C2, C3, One, Spec, Src0, Src1,
    _has_src1, _spill_c3_to_src1, lower, relu,
)
from concourse.dve_uop import DveOpSpec

F32 = mybir.dt.float32
F32R = mybir.dt.float32r
F16 = mybir.dt.float16
AOT = mybir.AluOpType
ACT_F = mybir.ActivationFunctionType

B_TOT, H, W = 32, 512, 512
N_CORES = 8
B_LOC = B_TOT // N_CORES           # images per core
Q = H // 2                          # quad-plane dim (256)
NPAGE = Q // 128                    # pages per quad plane (2)
DELTA = 255.0 / 16.0                # knot spacing of the LUT

# ---- tuning knobs ---------------------------------------------------------
OFFLOAD_C = 0          # how many trailing kinks go to ACT+PE (even number)
# engine for each of the 16 demosaic clip/evac tiles (8 outs x 2 pages)
DEM_CLIP_ENGINE = ["act"] * 16
# engine for each of the 8 raw clip tiles (4 quads x 2 pages)
RAW_CLIP_ENGINE = ["gpsimd"] * 8
HBLUR_STT_ENGINE = "vector"   # hblur combine (x = rho*s + cen); stt is DVE-only
HBLUR_PAIR_ENGINE = "gpsimd"  # hblur neighbor pair sum


# ---------------------------------------------------------------------------
# custom DVE ops (2 LUT segments per instruction)
# ---------------------------------------------------------------------------

def _head_ref(in0, in1, s0, s1, imm2):
    p = in0.shape[0]
    x = np.asarray(in0, np.float32).reshape(p, -1)
    d1 = np.asarray(in1, np.float32).reshape(p, 1)
    return (s0 * x + s1) + d1 * np.maximum(x - 1.0, 0.0)


def _pair_ref(in0, in1, s0, s1, imm2):
    x = np.asarray(in0, np.float32)
    acc = np.asarray(in1, np.float32).reshape(x.shape)
    return (acc + s0 * np.maximum(x - imm2, 0.0)) + s1 * np.maximum(
        x - imm2 - 1.0, 0.0
    )


def _register_op(name, spec):
    for op in _dops.OPS:
        if op.name == name:
            return op
    row = _dops._CUSTOM_DVE_ROW_BASE + len(_dops.OPS)
    assert row < 0x20, "custom DVE opcode rows exhausted"
    _dops._SUB_OPCODE_FOR_NAME[name] = row
    shas = {}
    for ver in ("v3", "v4"):
        try:
            s = DveOpSpec(name=name, opcode=row, uops=lower(spec, ver=ver),
                          rd1_en=_has_src1(spec))
            shas[ver] = s.sha(ver)
        except Exception:
            pass
    op = _dops.DveOp(name, spec, subdim=False, uops_sha=shas)
    _dops.OPS.append(op)
    _dops.CUSTOM_DVE_SPECS[name] = spec
    return op


# out = (C0*x + C1) + d1*relu(x - 1)   [affine + knot-1 term; d1 via in1]
INTERP_HEAD = _register_op(
    "CAM_INTERP_HEAD",
    Spec(body=_spill_c3_to_src1((C0 * Src0 + C1) + C3 * relu(Src0 - One)),
         reference=_head_ref),
)
# out = (acc + C0*relu(x - C2)) + C1*relu(x - C2 - 1)
INTERP_PAIR = _register_op(
    "CAM_INTERP_PAIR",
    Spec(body=(Src1 + C0 * relu(Src0 - C2)) + C1 * relu(Src0 - (C2 + One)),
         reference=_pair_ref),
)


# ---------------------------------------------------------------------------
# host-side constant planning
# ---------------------------------------------------------------------------

def _gauss1d(sigma=0.4):
    x = np.array([-1.0, 0.0, 1.0], dtype=np.float64)
    g = np.exp(-(x ** 2) / (2.0 * sigma ** 2))
    g /= g.sum()
    return g.astype(np.float32)  # [g0, g1, g0]


_G_AT = np.array([[0, 0, -1, 0, 0], [0, 0, 2, 0, 0], [-1, 2, 4, 2, -1],
                  [0, 0, 2, 0, 0], [0, 0, -1, 0, 0]], np.float32) / 8.0
_K_H = np.array([[0, 0, 0.5, 0, 0], [0, -1, 0, -1, 0], [-1, 4, 5, 4, -1],
                 [0, -1, 0, -1, 0], [0, 0, 0.5, 0, 0]], np.float32) / 8.0
_K_V = _K_H.T.copy()
_K_D = np.array([[0, 0, -1.5, 0, 0], [0, 2, 0, 2, 0], [-1.5, 0, 6, 0, -1.5],
                 [0, 2, 0, 2, 0], [0, 0, -1.5, 0, 0]], np.float32) / 8.0
FILTS = {"G": _G_AT, "H": _K_H, "V": _K_V, "D": _K_D}

# (out-quad (r,c), filter) -> output channel
CONV_OUT = [
    ((0, 0), "V", 0), ((0, 1), "D", 0),
    ((0, 0), "H", 2), ((0, 1), "G", 1),
    ((1, 0), "G", 1), ((1, 1), "H", 0),
    ((1, 0), "D", 2), ((1, 1), "V", 2),
]
RAW_OUT = {(0, 0): 1, (0, 1): 2, (1, 0): 0, (1, 1): 1}  # quad -> raw channel

# quad planes: 0=G1(even,even) 1=B(even,odd) 2=R(odd,even) 3=G2(odd,odd)
# source SBUF plane (index into the 4 loaded (ch,rowpar) planes) + col parity
# loaded planes: 0=Gev(ch1,r0) 1=Bev(ch2,r0) 2=Rod(ch0,r1) 3=God(ch1,r1)
PLANE_SRC = {0: (0, 0), 1: (1, 1), 2: (2, 0), 3: (3, 1)}
LOAD_SRCS = [(1, 0), (2, 0), (0, 1), (1, 1)]  # (im channel, row parity)
QUAD_CH = {0: 1, 1: 2, 2: 0, 3: 1}   # quad idx -> lut channel (G,B,R,G)


class _BandBuilder:
    def __init__(self):
        self.mats = []          # list of [128,128] float32
        self._idx = {}

    def add(self, m):
        key = m.tobytes()
        if key not in self._idx:
            self._idx[key] = len(self.mats)
            self.mats.append(m.copy())
        return self._idx[key]


def build_plan(yp):
    """All host-derived constants. yp: [3,17] float32 (255-domain)."""
    yp = np.asarray(yp, np.float32)
    g = _gauss1d()
    g0, g1 = float(g[0]), float(g[1])
    scale_v = 255.0 * g1 / DELTA          # folded into v-blur bands
    rho = g0 / g1                          # h-blur neighbor weight
    cd, co = scale_v * g1, scale_v * g0

    # ---- vblur bands (f32r): quad-domain 3-tap vertical blur -------------
    I = np.eye(128, dtype=np.float32)
    sub = np.zeros((128, 128), np.float32)   # band[k,m]: out m <- src k
    for m in range(1, 128):
        sub[m - 1, m] = 1.0                  # src row m-1
    sup = np.zeros((128, 128), np.float32)
    for m in range(127):
        sup[m + 1, m] = 1.0                  # src row m+1

    vb32 = _BandBuilder()
    b_cen = vb32.add(cd * I)
    m_up0 = co * (I + sub)
    m_up0[0, 0] += co                        # reflect: nei[-1] -> nei[0]
    b_up0 = vb32.add(m_up0)
    b_up1 = vb32.add(co * (I + sub))
    m_upx = np.zeros((128, 128), np.float32)
    m_upx[127, 0] = co                       # page1 m=0 <- page0 row127
    b_upx = vb32.add(m_upx)
    b_dn0 = vb32.add(co * (I + sup))
    m_dnx = np.zeros((128, 128), np.float32)
    m_dnx[0, 127] = co                       # page0 m=127 <- page1 row0
    b_dnx = vb32.add(m_dnx)
    m_dn1 = co * (I + sup)
    m_dn1[127, 127] += co                    # reflect at bottom edge
    b_dn1 = vb32.add(m_dn1)

    # fam = out-quad; (cen plane, nei plane, direction)
    vb_fams = {
        0: (0, 2, "up"), 1: (1, 3, "up"),
        2: (2, 0, "dn"), 3: (3, 1, "dn"),
    }
    # per (fam, page): list of (band_idx, plane, src_page)
    vb_mm = {}
    for qi, (cen, nei, d) in vb_fams.items():
        pages = []
        for pg in range(NPAGE):
            mm = [(b_cen, cen, pg)]
            if d == "up":
                if pg == 0:
                    mm.append((b_up0, nei, 0))
                else:
                    mm.append((b_up1, nei, 1))
                    mm.append((b_upx, nei, 0))
            else:
                if pg == 0:
                    mm.append((b_dn0, nei, 0))
                    mm.append((b_dnx, nei, 1))
                else:
                    mm.append((b_dn1, nei, 1))
            pages.append(mm)
        vb_mm[qi] = pages

    # ---- demosaic bands (fp16) -------------------------------------------
    bb = _BandBuilder()

    def reflected_src(i_src, pr):
        if 0 <= i_src < Q:
            return i_src
        y_src = 2 * i_src + pr
        y_r = -y_src if y_src < 0 else 2 * (H - 1) - y_src
        assert y_r % 2 == pr
        return y_r // 2

    def emit_groups(groups, page):
        mats = {}
        for (plane, sj), terms in sorted(groups.items()):
            for m in range(128):
                i_out = 128 * page + m
                for si, w, pr in terms:
                    i_src = reflected_src(i_out + si, pr)
                    sp, k = i_src // 128, i_src % 128
                    key = (plane, sp, sj)
                    if key not in mats:
                        mats[key] = np.zeros((128, 128), np.float32)
                    mats[key][k, m] += w
        return [(bb.add(mat), plane, sp, sj)
                for (plane, sp, sj), mat in sorted(mats.items(),
                                                   key=lambda x: x[0])]

    dem = []
    for (r, c), fname, ch in CONV_OUT:
        K = FILTS[fname]
        groups = {}
        for dy in range(-2, 3):
            for dx in range(-2, 3):
                w = float(K[2 + dy, 2 + dx])
                if w == 0.0:
                    continue
                pr = (r + dy) % 2
                si = (r + dy - pr) // 2
                pc = (c + dx) % 2
                sj = (c + dx - pc) // 2
                plane = 2 * pr + pc
                groups.setdefault((plane, sj), []).append((si, w, pr))
        pages = [emit_groups(groups, page) for page in range(NPAGE)]
        dem.append(((r, c), fname, ch, pages))

    # offload diag bands (fp16): per offloaded kink per lut channel d_k * I
    yps = yp / 255.0
    interp = []
    for ch in range(3):
        y0 = float(yps[ch, 0])
        s = np.diff(yps[ch]).astype(np.float64)   # 16 slopes per knot-unit
        d = np.diff(s)                            # 15 kink deltas (i=1..15)
        interp.append({
            "y0": y0, "s0": float(s[0]),
            "d": [float(v) for v in d],           # d[i-1] = kink at knot i
        })

    off_bands = {}    # (ch, kink) -> fp16 band idx
    if OFFLOAD_C > 0:
        for ch in range(3):
            for k in range(16 - OFFLOAD_C, 16):
                off_bands[(ch, k)] = bb.add(
                    np.float32(interp[ch]["d"][k - 1]) * I)
        ident16 = bb.add(I)
    else:
        ident16 = None

    bands32 = np.stack(vb32.mats).astype(np.float32)
    bands16 = np.stack(bb.mats).astype(np.float16)
    return {
        "bands32": bands32, "bands16": bands16,
        "vb_mm": vb_mm, "dem": dem, "interp": interp, "rho": rho,
        "off_bands": off_bands, "ident16": ident16,
    }


# ---------------------------------------------------------------------------
# kernel builder
# ---------------------------------------------------------------------------

def _row_decimated(ap2d, parity):
    """[512, 512] DRAM AP -> [128, NPAGE, 512] for rows parity::2."""
    rows = ap2d.rearrange("(r two) w -> two r w", two=2)[parity]
    return rows.rearrange("(q p) w -> p q w", p=128)


def _par_page(ap2d):
    """[512, 512] DRAM AP -> [128, 2(par), NPAGE, 512]."""
    return ap2d.rearrange("(q p t) w -> p t q w", p=128, t=2)


def build_kernel(plan):
    nc = bacc.Bacc(None, target_bir_lowering=False, debug=False)
    im = nc.dram_tensor("im", [B_LOC, 3, H, W], F32R,
                        kind="ExternalInput").ap()
    noise = nc.dram_tensor("noise", [B_LOC, 1, H, W], F16,
                           kind="ExternalInput").ap()
    n32 = plan["bands32"].shape[0]
    n16 = plan["bands16"].shape[0]
    bands32_d = nc.dram_tensor("bands32", [n32, 128, 128], F32R,
                               kind="ExternalInput").ap()
    bands16_d = nc.dram_tensor("bands16", [n16, 128, 128], F16,
                               kind="ExternalInput").ap()
    out = nc.dram_tensor("out", [B_LOC, 3, H, W], F16,
                         kind="ExternalOutput").ap()

    rho = plan["rho"]
    itp = plan["interp"]
    n_kink_dve = 15 - OFFLOAD_C        # kinks 1..n_kink_dve on DVE
    n_pairs = (n_kink_dve - 1) // 2
    assert (n_kink_dve - 1) % 2 == 0, "OFFLOAD_C must be even"

    from contextlib import ExitStack
    with tile.TileContext(nc) as tc, ExitStack() as ctx:
        consts = ctx.enter_context(tc.tile_pool(name="consts", bufs=1))
        imp = ctx.enter_context(tc.tile_pool(name="imp", bufs=2))
        nsp = ctx.enter_context(tc.tile_pool(name="nsp", bufs=2))
        sxp = ctx.enter_context(tc.tile_pool(name="sxp", bufs=2))
        xtp = ctx.enter_context(tc.tile_pool(name="xtp", bufs=2))
        accp = ctx.enter_context(tc.tile_pool(name="accp", bufs=2))
        nyr = ctx.enter_context(tc.tile_pool(name="nyr", bufs=2))
        rlp = ctx.enter_context(tc.tile_pool(name="rlp", bufs=2))
        cvp = ctx.enter_context(tc.tile_pool(name="cvp", bufs=4))
        outp = ctx.enter_context(tc.tile_pool(name="outp", bufs=2))
        psum_vb = ctx.enter_context(
            tc.tile_pool(name="psvb", bufs=2, space="PSUM"))
        psum_ac = ctx.enter_context(
            tc.tile_pool(name="psac", bufs=2, space="PSUM"))
        psum_dm = ctx.enter_context(
            tc.tile_pool(name="psdm", bufs=4, space="PSUM"))

        # --- constants ---
        b32_all = consts.tile([128, n32, 128], F32R, tag="bands32")
        nc.sync.dma_start(out=b32_all,
                          in_=bands32_d.rearrange("n k m -> k n m"))
        band32_t = [b32_all[:, i, :] for i in range(n32)]
        b16_all = consts.tile([128, n16, 128], F16, tag="bands16")
        nc.sync.dma_start(out=b16_all,
                          in_=bands16_d.rearrange("n k m -> k n m"))
        band16_t = [b16_all[:, i, :] for i in range(n16)]
        d1_t = consts.tile([128, 3], F32, tag="d1")
        for ch in range(3):
            nc.vector.memset(d1_t[:, ch:ch + 1], itp[ch]["d"][0])

        def engine_of(name):
            return {"act": nc.scalar, "gpsimd": nc.gpsimd,
                    "vector": nc.vector}[name]

        def emit_front(b):
            """DMA loads for image b."""
            ptiles = []
            for pi, (ch, par) in enumerate(LOAD_SRCS):
                t = imp.tile([128, NPAGE, W], F32R, tag=f"plane{pi}",
                             name=f"pl{b}_{pi}")
                nc.sync.dma_start(out=t, in_=_row_decimated(im[b, ch], par))
                ptiles.append(t)
            noi = nsp.tile([128, 2, NPAGE, W], F16, tag="noise",
                           name=f"ns{b}")
            nc.sync.dma_start(out=noi, in_=_par_page(noise[b, 0]))
            return ptiles, noi

        def emit_image(b, front):
            ptiles, noi = front

            # --- vblur (PE, quad domain, free dim 256) ---
            # GPSIMD can't read PSUM, so evacuate via ACT copy to SBUF.
            vbt = {}
            for qi in range(4):
                ps = psum_vb.tile([128, NPAGE, Q], F32,
                                  tag=f"vb{'AB'[qi % 2]}",
                                  name=f"vps{b}_{qi}")
                for pg, mm in enumerate(plan["vb_mm"][qi]):
                    for i, (bidx, plane, spage) in enumerate(mm):
                        src_pi, cpar = PLANE_SRC[plane]
                        nc.tensor.matmul(
                            ps[:, pg, :], band32_t[bidx],
                            ptiles[src_pi][:, spage, cpar:W:2],
                            start=(i == 0), stop=(i == len(mm) - 1))
                vs = sxp.tile([128, NPAGE, Q], F32, tag=f"vbs{qi}",
                              name=f"vbs{b}_{qi}")
                nc.scalar.copy(out=vs[:], in_=ps[:])
                vbt[qi] = vs

            # --- hblur: x = cen + rho * (neighbor pair sum) ---
            # x tiles: G chain holds quads 0 (pages 0:2) and 3 (pages 2:4)
            x_G = xtp.tile([128, 2 * NPAGE, Q], F32, tag="xG",
                           name=f"xG{b}")
            x_B = xtp.tile([128, NPAGE, Q], F32, tag="xB", name=f"xB{b}")
            x_R = xtp.tile([128, NPAGE, Q], F32, tag="xR", name=f"xR{b}")
            x_of = {0: x_G[:, 0:NPAGE, :], 3: x_G[:, NPAGE:2 * NPAGE, :],
                    1: x_B[:], 2: x_R[:]}
            pair_eng = engine_of(HBLUR_PAIR_ENGINE)
            stt_eng = engine_of(HBLUR_STT_ENGINE)
            for qi, (r, c) in enumerate(((0, 0), (0, 1), (1, 0), (1, 1))):
                cen = vbt[qi]
                nei = vbt[2 * r + (1 - c)]
                s = sxp.tile([128, NPAGE, Q], F32, tag="s",
                             name=f"s{b}_{qi}")
                if c == 0:
                    pair_eng.tensor_add(out=s[:, :, 1:Q],
                                        in0=nei[:, :, 0:Q - 1],
                                        in1=nei[:, :, 1:Q])
                    nc.vector.tensor_scalar_mul(
                        out=s[:, :, 0:1], in0=nei[:, :, 0:1], scalar1=2.0)
                else:
                    pair_eng.tensor_add(out=s[:, :, 0:Q - 1],
                                        in0=nei[:, :, 0:Q - 1],
                                        in1=nei[:, :, 1:Q])
                    nc.vector.tensor_scalar_mul(
                        out=s[:, :, Q - 1:Q], in0=nei[:, :, Q - 1:Q],
                        scalar1=2.0)
                stt_eng.scalar_tensor_tensor(
                    out=x_of[qi], in0=s, scalar=rho, in1=cen[:],
                    op0=AOT.mult, op1=AOT.add)

            # --- interp chains (custom DVE; 2 kinks per instruction) ---
            def interp_chain(ch, xt, shape_free, tagc):
                co = itp[ch]
                xf = xt.rearrange("p a b -> p (a b)")
                a0 = accp.tile([128, shape_free, Q], F32, tag=f"a0{tagc}",
                               name=f"a0{b}{tagc}")
                a1 = accp.tile([128, shape_free, Q], F32, tag=f"a1{tagc}",
                               name=f"a1{b}{tagc}")
                nc.vector._custom_dve(
                    INTERP_HEAD, out=a0[:].rearrange("p a b -> p (a b)"),
                    in0=xf, in1=d1_t[:, ch:ch + 1],
                    s0=co["s0"], s1=co["y0"])
                src, dst = a0, a1
                for j in range(1, n_pairs + 1):
                    nc.vector._custom_dve(
                        INTERP_PAIR,
                        out=dst[:].rearrange("p a b -> p (a b)"),
                        in0=xf,
                        in1=src[:].rearrange("p a b -> p (a b)"),
                        s0=co["d"][2 * j - 1], s1=co["d"][2 * j],
                        imm2=float(2 * j))
                    src, dst = dst, src
                return src

            acc_G = interp_chain(1, x_G[:], 2 * NPAGE, "G")
            acc_B = interp_chain(2, x_B[:], NPAGE, "B")
            acc_R = interp_chain(0, x_R[:], NPAGE, "R")
            acc_of = {0: acc_G[:, 0:NPAGE, :], 3: acc_G[:, NPAGE:2 * NPAGE, :],
                      1: acc_B[:], 2: acc_R[:]}

            # --- offloaded kinks: ACT relu planes + PE diag accumulation ---
            pacc_of = {}
            if OFFLOAD_C > 0:
                # P_acc[quad] starts with the noise (identity band)
                for qi, (r, c) in enumerate(((0, 0), (0, 1), (1, 0), (1, 1))):
                    pa = psum_ac.tile([128, NPAGE, Q], F32, tag="pacc",
                                      name=f"pa{b}_{qi}")
                    for pg in range(NPAGE):
                        nc.tensor.matmul(
                            pa[:, pg, :], band16_t[plan["ident16"]],
                            noi[:, r, pg, c:W:2],
                            start=True, stop=False)
                    pacc_of[qi] = pa
                for k in range(16 - OFFLOAD_C, 16):
                    r_G = rlp.tile([128, 2 * NPAGE, Q], F16, tag="rG",
                                   name=f"rG{b}_{k}")
                    r_B = rlp.tile([128, NPAGE, Q], F16, tag="rB",
                                   name=f"rB{b}_{k}")
                    r_R = rlp.tile([128, NPAGE, Q], F16, tag="rR",
                                   name=f"rR{b}_{k}")
                    nc.scalar.activation(out=r_G[:], in_=x_G[:],
                                         func=ACT_F.Relu, bias=-float(k))
                    nc.scalar.activation(out=r_B[:], in_=x_B[:],
                                         func=ACT_F.Relu, bias=-float(k))
                    nc.scalar.activation(out=r_R[:], in_=x_R[:],
                                         func=ACT_F.Relu, bias=-float(k))
                    r_of = {0: r_G[:, 0:NPAGE, :],
                            3: r_G[:, NPAGE:2 * NPAGE, :],
                            1: r_B[:], 2: r_R[:]}
                    last = (k == 15)
                    for qi in range(4):
                        bidx = plan["off_bands"][(QUAD_CH[qi], k)]
                        for pg in range(NPAGE):
                            nc.tensor.matmul(
                                pacc_of[qi][:, pg, :], band16_t[bidx],
                                r_of[qi][:, pg, :],
                                start=False, stop=(last and True))

            # --- nyr = interp + noise (+ offloaded part), fp16, col halo ---
            nyrtiles = []
            for qi, (r, c) in enumerate(((0, 0), (0, 1), (1, 0), (1, 1))):
                npr = nyr.tile([128, NPAGE, Q + 2], F16, tag=f"nyr{qi}",
                               name=f"npr{b}_{qi}")
                if OFFLOAD_C > 0:
                    # Pool can't read PSUM; use DVE for the PSUM+SBUF combine
                    nc.vector.scalar_tensor_tensor(
                        out=npr[:, :, 1:Q + 1], in0=pacc_of[qi][:],
                        scalar=1.0, in1=acc_of[qi],
                        op0=AOT.mult, op1=AOT.add)
                else:
                    nc.gpsimd.tensor_add(
                        out=npr[:, :, 1:Q + 1],
                        in0=acc_of[qi],
                        in1=noi[:, r, :, c:W:2])
                lsrc = 2 if c == 0 else 1
                rsrc = Q if c == 0 else Q - 1
                nc.gpsimd.tensor_copy(out=npr[:, :, 0:1],
                                      in_=npr[:, :, lsrc:lsrc + 1])
                nc.gpsimd.tensor_copy(out=npr[:, :, Q + 1:Q + 2],
                                      in_=npr[:, :, rsrc:rsrc + 1])
                nyrtiles.append(npr)

            # --- output tiles (fp16) ---
            ot = [outp.tile([128, 2, NPAGE, W], F16, tag=f"o{ch}",
                            name=f"ot{b}_{ch}") for ch in range(3)]

            # raw channel clip
            for qi, (r, c) in enumerate(((0, 0), (0, 1), (1, 0), (1, 1))):
                rch = RAW_OUT[(r, c)]
                for pg in range(NPAGE):
                    eng = engine_of(RAW_CLIP_ENGINE[qi * NPAGE + pg])
                    eng.tensor_scalar(
                        out=ot[rch][:, r, pg, c:W:2],
                        in0=nyrtiles[qi][:, pg, 1:Q + 1],
                        scalar1=0.0, scalar2=1.0,
                        op0=AOT.max, op1=AOT.min)

            # --- demosaic (PE fp16) + clip/evac ---
            ci = 0
            for (r, c), fname, ch, pages in plan["dem"]:
                for page, mains in enumerate(pages):
                    ps = psum_dm.tile([128, Q], F32, tag="dmps",
                                      name=f"dm{b}_{ci}")
                    for i, (bidx, plane, spage, sj) in enumerate(mains):
                        nc.tensor.matmul(
                            ps[:], band16_t[bidx],
                            nyrtiles[plane][:, spage, 1 + sj:1 + sj + Q],
                            start=(i == 0), stop=(i == len(mains) - 1))
                    dst = ot[ch][:, r, page, c:W:2]
                    ename = DEM_CLIP_ENGINE[ci]
                    if ename == "act":
                        tcl = cvp.tile([128, Q], F32, tag="conv",
                                       name=f"cv{b}_{ci}")
                        nc.scalar.activation(out=tcl[:], in_=ps[:],
                                             func=ACT_F.Relu,
                                             scale=-1.0, bias=1.0)
                        nc.scalar.activation(out=dst, in_=tcl[:],
                                             func=ACT_F.Relu,
                                             scale=-1.0, bias=1.0)
                    else:
                        engine_of(ename).tensor_scalar(
                            out=dst, in0=ps[:],
                            scalar1=0.0, scalar2=1.0,
                            op0=AOT.max, op1=AOT.min)
                    ci += 1

            # --- stores (3 DMAs) ---
            for ch in range(3):
                nc.sync.dma_start(out=_par_page(out[b, ch]), in_=ot[ch][:])

        fronts = [emit_front(0), emit_front(1)]
        for b in range(B_LOC):
            if b + 2 < B_LOC:
                fronts.append(emit_front(b + 2))
            emit_image(b, fronts[b])

    nc.compile()
    return nc


# ---------------------------------------------------------------------------
# public entry
# ---------------------------------------------------------------------------

_CACHE = {}


def _get_compiled(yp):
    key = np.asarray(yp, np.float32).tobytes()
    if key not in _CACHE:
        plan = build_plan(yp)
        _CACHE[key] = (build_kernel(plan), plan)
    return _CACHE[key]


def build_in_maps(im, yp, noise):
    im = np.ascontiguousarray(np.asarray(im, np.float32))
    noise = np.asarray(noise, np.float32)
    nc, plan = _get_compiled(np.asarray(yp, np.float32))
    noise_s = np.ascontiguousarray(
        (noise * np.float32(1.0 / 255.0)).astype(np.float16))
    in_maps = []
    for k in range(N_CORES):
        sl = slice(k * B_LOC, (k + 1) * B_LOC)
        in_maps.append({
            "im": im[sl],
            "noise": noise_s[sl],
            "bands32": plan["bands32"],
            "bands16": plan["bands16"],
        })
    return nc, in_maps


def kernel(im, yp, noise):
    nc, in_maps = build_in_maps(im, yp, noise)
    res = run_bass_kernel_spmd(nc, in_maps, core_ids=list(range(N_CORES)))
    return np.concatenate(
        [np.asarray(r["out"], np.float16).astype(np.float32)
         for r in res.results], axis=0)


# revision 10
# speedup vs baseline: 1.0118x; 1.0118x over previous
"""Trainium2 Bass kernel for the Camera ISP pipeline (mosaic -> gaussian blur
-> per-channel piecewise-linear calibration -> noise -> Malvar demosaic -> clip).

v3 strategy (per core, pure data parallel over batch: 4 images/core):
- Row-decimated contiguous DMA loads of the 4 bayer source planes (f32r,
  full width); noise and output travel as fp16 (halves their DMA bytes).
- Vertical blur directly in the quad (polyphase) domain: banded matmuls on
  PE with free dim 256 (only the column parity each family needs), ACT
  evacuation, horizontal blur as Pool pair-sum + DVE combine.
- 17-knot np.interp: chained custom DVE instructions (2 piecewise segments
  each); G quads share one chain. The last OFFLOAD_C kinks run as ACT relu
  planes accumulated by PE diagonal bands into PSUM together with the noise
  and the DVE part (identity bands), so the noisy linear image (nyr) is
  assembled in PSUM and evacuated once by ACT.
- Malvar 5x5 demosaic as fp16 banded matmuls accumulating in PSUM.
- Clip/evac: ACT relu (PSUM->SBUF) + Pool min, writing fp16 output tiles;
  3 output DMAs per image issued from ACT's queue so SP's input-load queue
  never blocks behind them.
- Software-pipelined emission: the next image's blur front is emitted
  between this image's interp chains and its demosaic, so PE/ACT/Pool keep
  working while DVE runs the serial chains.
"""

import sys

sys.path.insert(0, "/opt/trn_rl_repo")

import numpy as np

import concourse.bacc as bacc
import concourse.bass as bass
import concourse.tile as tile
from concourse import mybir
from concourse.bass_utils import run_bass_kernel_spmd
from concourse import dve_ops as _dops
from concourse.dve_spec import (
    C0, C1, C2, C3, One, Spec, Src0, Src1,
    _has_src1, _spill_c3_to_src1, lower, relu,
)
from concourse.dve_uop import DveOpSpec

F32 = mybir.dt.float32
F32R = mybir.dt.float32r
F16 = mybir.dt.float16
AOT = mybir.AluOpType
ACT_F = mybir.ActivationFunctionType

B_TOT, H, W = 32, 512, 512
N_CORES = 8
B_LOC = B_TOT // N_CORES           # images per core
Q = H // 2                          # quad-plane dim (256)
NPAGE = Q // 128                    # pages per quad plane (2)
DELTA = 255.0 / 16.0                # knot spacing of the LUT

# ---- tuning knobs ---------------------------------------------------------
OFFLOAD_C = 2          # trailing kinks on ACT+PE instead of DVE (even)
# engine for the second clip op (min) of the 16 demosaic tiles
DEM_MIN_ENGINE = ["gpsimd"] * 16
# engine for each of the 8 raw clip tiles
RAW_CLIP_ENGINE = ["gpsimd"] * 8


# ---------------------------------------------------------------------------
# custom DVE ops (2 LUT segments per instruction)
# ---------------------------------------------------------------------------

def _head_ref(in0, in1, s0, s1, imm2):
    p = in0.shape[0]
    x = np.asarray(in0, np.float32).reshape(p, -1)
    d1 = np.asarray(in1, np.float32).reshape(p, 1)
    return (s0 * x + s1) + d1 * np.maximum(x - 1.0, 0.0)


def _pair_ref(in0, in1, s0, s1, imm2):
    x = np.asarray(in0, np.float32)
    acc = np.asarray(in1, np.float32).reshape(x.shape)
    return (acc + s0 * np.maximum(x - imm2, 0.0)) + s1 * np.maximum(
        x - imm2 - 1.0, 0.0
    )


def _register_op(name, spec):
    for op in _dops.OPS:
        if op.name == name:
            return op
    row = _dops._CUSTOM_DVE_ROW_BASE + len(_dops.OPS)
    assert row < 0x20, "custom DVE opcode rows exhausted"
    _dops._SUB_OPCODE_FOR_NAME[name] = row
    shas = {}
    for ver in ("v3", "v4"):
        try:
            s = DveOpSpec(name=name, opcode=row, uops=lower(spec, ver=ver),
                          rd1_en=_has_src1(spec))
            shas[ver] = s.sha(ver)
        except Exception:
            pass
    op = _dops.DveOp(name, spec, subdim=False, uops_sha=shas)
    _dops.OPS.append(op)
    _dops.CUSTOM_DVE_SPECS[name] = spec
    return op


INTERP_HEAD = _register_op(
    "CAM_INTERP_HEAD",
    Spec(body=_spill_c3_to_src1((C0 * Src0 + C1) + C3 * relu(Src0 - One)),
         reference=_head_ref),
)
INTERP_PAIR = _register_op(
    "CAM_INTERP_PAIR",
    Spec(body=(Src1 + C0 * relu(Src0 - C2)) + C1 * relu(Src0 - (C2 + One)),
         reference=_pair_ref),
)


# ---------------------------------------------------------------------------
# host-side constant planning
# ---------------------------------------------------------------------------

def _gauss1d(sigma=0.4):
    x = np.array([-1.0, 0.0, 1.0], dtype=np.float64)
    g = np.exp(-(x ** 2) / (2.0 * sigma ** 2))
    g /= g.sum()
    return g.astype(np.float32)  # [g0, g1, g0]


_G_AT = np.array([[0, 0, -1, 0, 0], [0, 0, 2, 0, 0], [-1, 2, 4, 2, -1],
                  [0, 0, 2, 0, 0], [0, 0, -1, 0, 0]], np.float32) / 8.0
_K_H = np.array([[0, 0, 0.5, 0, 0], [0, -1, 0, -1, 0], [-1, 4, 5, 4, -1],
                 [0, -1, 0, -1, 0], [0, 0, 0.5, 0, 0]], np.float32) / 8.0
_K_V = _K_H.T.copy()
_K_D = np.array([[0, 0, -1.5, 0, 0], [0, 2, 0, 2, 0], [-1.5, 0, 6, 0, -1.5],
                 [0, 2, 0, 2, 0], [0, 0, -1.5, 0, 0]], np.float32) / 8.0
FILTS = {"G": _G_AT, "H": _K_H, "V": _K_V, "D": _K_D}

CONV_OUT = [
    ((0, 0), "V", 0), ((0, 1), "D", 0),
    ((0, 0), "H", 2), ((0, 1), "G", 1),
    ((1, 0), "G", 1), ((1, 1), "H", 0),
    ((1, 0), "D", 2), ((1, 1), "V", 2),
]
RAW_OUT = {(0, 0): 1, (0, 1): 2, (1, 0): 0, (1, 1): 1}  # quad -> raw channel

# quad planes: 0=G1(even,even) 1=B(even,odd) 2=R(odd,even) 3=G2(odd,odd)
PLANE_SRC = {0: (0, 0), 1: (1, 1), 2: (2, 0), 3: (3, 1)}
LOAD_SRCS = [(1, 0), (2, 0), (0, 1), (1, 1)]  # (im channel, row parity)
QUAD_CH = {0: 1, 1: 2, 2: 0, 3: 1}   # quad idx -> lut channel (G,B,R,G)
QUADS = ((0, 0), (0, 1), (1, 0), (1, 1))


class _BandBuilder:
    def __init__(self):
        self.mats = []
        self._idx = {}

    def add(self, m):
        key = m.tobytes()
        if key not in self._idx:
            self._idx[key] = len(self.mats)
            self.mats.append(m.copy())
        return self._idx[key]


def build_plan(yp):
    """All host-derived constants. yp: [3,17] float32 (255-domain)."""
    yp = np.asarray(yp, np.float32)
    g = _gauss1d()
    g0, g1 = float(g[0]), float(g[1])
    scale_v = 255.0 * g1 / DELTA
    rho = g0 / g1
    cd, co = scale_v * g1, scale_v * g0

    I = np.eye(128, dtype=np.float32)
    sub = np.zeros((128, 128), np.float32)
    for m in range(1, 128):
        sub[m - 1, m] = 1.0
    sup = np.zeros((128, 128), np.float32)
    for m in range(127):
        sup[m + 1, m] = 1.0

    vb32 = _BandBuilder()
    b_cen = vb32.add(cd * I)
    m_up0 = co * (I + sub)
    m_up0[0, 0] += co
    b_up0 = vb32.add(m_up0)
    b_up1 = vb32.add(co * (I + sub))
    m_upx = np.zeros((128, 128), np.float32)
    m_upx[127, 0] = co
    b_upx = vb32.add(m_upx)
    b_dn0 = vb32.add(co * (I + sup))
    m_dnx = np.zeros((128, 128), np.float32)
    m_dnx[0, 127] = co
    b_dnx = vb32.add(m_dnx)
    m_dn1 = co * (I + sup)
    m_dn1[127, 127] += co
    b_dn1 = vb32.add(m_dn1)
    ident32 = vb32.add(I)

    vb_fams = {
        0: (0, 2, "up"), 1: (1, 3, "up"),
        2: (2, 0, "dn"), 3: (3, 1, "dn"),
    }
    vb_mm = {}
    for qi, (cen, nei, d) in vb_fams.items():
        pages = []
        for pg in range(NPAGE):
            mm = [(b_cen, cen, pg)]
            if d == "up":
                if pg == 0:
                    mm.append((b_up0, nei, 0))
                else:
                    mm.append((b_up1, nei, 1))
                    mm.append((b_upx, nei, 0))
            else:
                if pg == 0:
                    mm.append((b_dn0, nei, 0))
                    mm.append((b_dnx, nei, 1))
                else:
                    mm.append((b_dn1, nei, 1))
            pages.append(mm)
        vb_mm[qi] = pages

    bb = _BandBuilder()

    def reflected_src(i_src, pr):
        if 0 <= i_src < Q:
            return i_src
        y_src = 2 * i_src + pr
        y_r = -y_src if y_src < 0 else 2 * (H - 1) - y_src
        assert y_r % 2 == pr
        return y_r // 2

    def emit_groups(groups, page):
        mats = {}
        for (plane, sj), terms in sorted(groups.items()):
            for m in range(128):
                i_out = 128 * page + m
                for si, w, pr in terms:
                    i_src = reflected_src(i_out + si, pr)
                    sp, k = i_src // 128, i_src % 128
                    key = (plane, sp, sj)
                    if key not in mats:
                        mats[key] = np.zeros((128, 128), np.float32)
                    mats[key][k, m] += w
        return [(bb.add(mat), plane, sp, sj)
                for (plane, sp, sj), mat in sorted(mats.items(),
                                                   key=lambda x: x[0])]

    dem = []
    for (r, c), fname, ch in CONV_OUT:
        K = FILTS[fname]
        groups = {}
        for dy in range(-2, 3):
            for dx in range(-2, 3):
                w = float(K[2 + dy, 2 + dx])
                if w == 0.0:
                    continue
                pr = (r + dy) % 2
                si = (r + dy - pr) // 2
                pc = (c + dx) % 2
                sj = (c + dx - pc) // 2
                plane = 2 * pr + pc
                groups.setdefault((plane, sj), []).append((si, w, pr))
        pages = [emit_groups(groups, page) for page in range(NPAGE)]
        dem.append(((r, c), fname, ch, pages))

    yps = yp / 255.0
    interp = []
    for ch in range(3):
        y0 = float(yps[ch, 0])
        s = np.diff(yps[ch]).astype(np.float64)
        d = np.diff(s)
        interp.append({
            "y0": y0, "s0": float(s[0]),
            "d": [float(v) for v in d],
        })

    off_bands = {}
    ident16 = bb.add(I)
    for ch in range(3):
        for k in range(16 - OFFLOAD_C, 16):
            off_bands[(ch, k)] = bb.add(
                np.float32(interp[ch]["d"][k - 1]) * I)

    bands32 = np.stack(vb32.mats).astype(np.float32)
    bands16 = np.stack(bb.mats).astype(np.float16)
    return {
        "bands32": bands32, "bands16": bands16,
        "vb_mm": vb_mm, "dem": dem, "interp": interp, "rho": rho,
        "off_bands": off_bands, "ident16": ident16, "ident32": ident32,
    }


# ---------------------------------------------------------------------------
# kernel builder
# ---------------------------------------------------------------------------

def _row_decimated(ap2d, parity):
    rows = ap2d.rearrange("(r two) w -> two r w", two=2)[parity]
    return rows.rearrange("(q p) w -> p q w", p=128)


def _par_page(ap2d):
    """[512, 512] DRAM AP -> [128, 2(par), NPAGE, 512]."""
    return ap2d.rearrange("(q p t) w -> p t q w", p=128, t=2)


def build_kernel(plan):
    nc = bacc.Bacc(None, target_bir_lowering=False, debug=False)
    im = nc.dram_tensor("im", [B_LOC, 3, H, W], F32R,
                        kind="ExternalInput").ap()
    noise = nc.dram_tensor("noise", [B_LOC, 1, H, W], F16,
                           kind="ExternalInput").ap()
    n32 = plan["bands32"].shape[0]
    n16 = plan["bands16"].shape[0]
    bands32_d = nc.dram_tensor("bands32", [n32, 128, 128], F32R,
                               kind="ExternalInput").ap()
    bands16_d = nc.dram_tensor("bands16", [n16, 128, 128], F16,
                               kind="ExternalInput").ap()
    out = nc.dram_tensor("out", [B_LOC, 3, H, W], F16,
                         kind="ExternalOutput").ap()

    rho = plan["rho"]
    itp = plan["interp"]
    n_kink_dve = 15 - OFFLOAD_C
    assert (n_kink_dve - 1) % 2 == 0, "OFFLOAD_C must be even"
    n_pairs = (n_kink_dve - 1) // 2

    from contextlib import ExitStack
    with tile.TileContext(nc) as tc, ExitStack() as ctx:
        consts = ctx.enter_context(tc.tile_pool(name="consts", bufs=1))
        imp = ctx.enter_context(tc.tile_pool(name="imp", bufs=2))
        nsp = ctx.enter_context(tc.tile_pool(name="nsp", bufs=2))
        sxp = ctx.enter_context(tc.tile_pool(name="sxp", bufs=2))
        xtp = ctx.enter_context(tc.tile_pool(name="xtp", bufs=2))
        accp = ctx.enter_context(tc.tile_pool(name="accp", bufs=2))
        nyr = ctx.enter_context(tc.tile_pool(name="nyr", bufs=2))
        rlp = ctx.enter_context(tc.tile_pool(name="rlp", bufs=2))
        cvp = ctx.enter_context(tc.tile_pool(name="cvp", bufs=4))
        outp = ctx.enter_context(tc.tile_pool(name="outp", bufs=2))
        psum_vb = ctx.enter_context(
            tc.tile_pool(name="psvb", bufs=1, space="PSUM"))
        psum_ac = ctx.enter_context(
            tc.tile_pool(name="psac", bufs=4, space="PSUM"))
        psum_dm = ctx.enter_context(
            tc.tile_pool(name="psdm", bufs=2, space="PSUM"))

        # --- constants ---
        b32_all = consts.tile([128, n32, 128], F32R, tag="bands32")
        nc.sync.dma_start(out=b32_all,
                          in_=bands32_d.rearrange("n k m -> k n m"))
        band32_t = [b32_all[:, i, :] for i in range(n32)]
        b16_all = consts.tile([128, n16, 128], F16, tag="bands16")
        nc.sync.dma_start(out=b16_all,
                          in_=bands16_d.rearrange("n k m -> k n m"))
        band16_t = [b16_all[:, i, :] for i in range(n16)]
        d1_t = consts.tile([128, 3], F32, tag="d1")
        for ch in range(3):
            nc.vector.memset(d1_t[:, ch:ch + 1], itp[ch]["d"][0])
        kb_t = consts.tile([128, max(OFFLOAD_C, 1)], F32, tag="kbias")
        for i, k in enumerate(range(16 - OFFLOAD_C, 16)):
            nc.vector.memset(kb_t[:, i:i + 1], -float(k))

        def engine_of(name):
            return {"act": nc.scalar, "gpsimd": nc.gpsimd,
                    "vector": nc.vector}[name]

        def emit_front(b):
            ptiles = []
            for pi, (ch, par) in enumerate(LOAD_SRCS):
                t = imp.tile([128, NPAGE, W], F32R, tag=f"plane{pi}",
                             name=f"pl{b}_{pi}")
                nc.sync.dma_start(out=t, in_=_row_decimated(im[b, ch], par))
                ptiles.append(t)
            noi = nsp.tile([128, 2, NPAGE, W], F16, tag="noise",
                           name=f"ns{b}")
            nc.sync.dma_start(out=noi, in_=_par_page(noise[b, 0]))
            return ptiles, noi

        def emit_blur(b, front):
            """vblur (PE) -> evac (ACT) -> hblur (Pool pair + DVE stt)."""
            ptiles, _noi = front
            vbt = {}
            for qi in range(4):
                ps = psum_vb.tile([128, NPAGE, Q], F32,
                                  tag=f"vb{'AB'[qi % 2]}",
                                  name=f"vps{b}_{qi}")
                for pg, mm in enumerate(plan["vb_mm"][qi]):
                    for i, (bidx, plane, spage) in enumerate(mm):
                        src_pi, cpar = PLANE_SRC[plane]
                        nc.tensor.matmul(
                            ps[:, pg, :], band32_t[bidx],
                            ptiles[src_pi][:, spage, cpar:W:2],
                            start=(i == 0), stop=(i == len(mm) - 1))
                vs = sxp.tile([128, NPAGE, Q], F32, tag=f"vbs{qi}",
                              name=f"vbs{b}_{qi}")
                nc.scalar.copy(out=vs[:], in_=ps[:])
                vbt[qi] = vs

            x_G = xtp.tile([128, 2 * NPAGE, Q], F32, tag="xG", name=f"xG{b}")
            x_B = xtp.tile([128, NPAGE, Q], F32, tag="xB", name=f"xB{b}")
            x_R = xtp.tile([128, NPAGE, Q], F32, tag="xR", name=f"xR{b}")
            x_of = {0: x_G[:, 0:NPAGE, :], 3: x_G[:, NPAGE:2 * NPAGE, :],
                    1: x_B[:], 2: x_R[:]}
            for qi, (r, c) in enumerate(QUADS):
                cen = vbt[qi]
                nei = vbt[2 * r + (1 - c)]
                s = sxp.tile([128, NPAGE, Q], F32, tag="s", name=f"s{b}_{qi}")
                if c == 0:
                    nc.gpsimd.tensor_add(out=s[:, :, 1:Q],
                                         in0=nei[:, :, 0:Q - 1],
                                         in1=nei[:, :, 1:Q])
                    nc.gpsimd.tensor_scalar_mul(
                        out=s[:, :, 0:1], in0=nei[:, :, 0:1], scalar1=2.0)
                else:
                    nc.gpsimd.tensor_add(out=s[:, :, 0:Q - 1],
                                         in0=nei[:, :, 0:Q - 1],
                                         in1=nei[:, :, 1:Q])
                    nc.gpsimd.tensor_scalar_mul(
                        out=s[:, :, Q - 1:Q], in0=nei[:, :, Q - 1:Q],
                        scalar1=2.0)
                nc.vector.scalar_tensor_tensor(
                    out=x_of[qi], in0=s, scalar=rho, in1=cen[:],
                    op0=AOT.mult, op1=AOT.add)
            return {"x_G": x_G, "x_B": x_B, "x_R": x_R, "x_of": x_of}

        def emit_chains(b, blur):
            """Custom-DVE interp chains + ACT relu planes for offload."""
            def interp_chain(ch, xt, shape_free, tagc):
                co = itp[ch]
                xf = xt.rearrange("p a b -> p (a b)")
                a0 = accp.tile([128, shape_free, Q], F32R, tag=f"a0{tagc}",
                               name=f"a0{b}{tagc}")
                a1 = accp.tile([128, shape_free, Q], F32R, tag=f"a1{tagc}",
                               name=f"a1{b}{tagc}")
                nc.vector._custom_dve(
                    INTERP_HEAD, out=a0[:].rearrange("p a b -> p (a b)"),
                    in0=xf, in1=d1_t[:, ch:ch + 1],
                    s0=co["s0"], s1=co["y0"])
                src, dst = a0, a1
                for j in range(1, n_pairs + 1):
                    nc.vector._custom_dve(
                        INTERP_PAIR,
                        out=dst[:].rearrange("p a b -> p (a b)"),
                        in0=xf,
                        in1=src[:].rearrange("p a b -> p (a b)"),
                        s0=co["d"][2 * j - 1], s1=co["d"][2 * j],
                        imm2=float(2 * j))
                    src, dst = dst, src
                return src

            acc_G = interp_chain(1, blur["x_G"][:], 2 * NPAGE, "G")
            acc_B = interp_chain(2, blur["x_B"][:], NPAGE, "B")
            acc_R = interp_chain(0, blur["x_R"][:], NPAGE, "R")
            acc_of = {0: acc_G[:, 0:NPAGE, :],
                      3: acc_G[:, NPAGE:2 * NPAGE, :],
                      1: acc_B[:], 2: acc_R[:]}

            relus = {}
            for k in range(16 - OFFLOAD_C, 16):
                r_G = rlp.tile([128, 2 * NPAGE, Q], F16, tag="rG",
                               name=f"rG{b}_{k}")
                r_B = rlp.tile([128, NPAGE, Q], F16, tag="rB",
                               name=f"rB{b}_{k}")
                r_R = rlp.tile([128, NPAGE, Q], F16, tag="rR",
                               name=f"rR{b}_{k}")
                kb = kb_t[:, k - (16 - OFFLOAD_C):k - (16 - OFFLOAD_C) + 1]
                nc.scalar.activation(out=r_G[:], in_=blur["x_G"][:],
                                     func=ACT_F.Relu, bias=kb)
                nc.scalar.activation(out=r_B[:], in_=blur["x_B"][:],
                                     func=ACT_F.Relu, bias=kb)
                nc.scalar.activation(out=r_R[:], in_=blur["x_R"][:],
                                     func=ACT_F.Relu, bias=kb)
                relus[k] = {0: r_G[:, 0:NPAGE, :],
                            3: r_G[:, NPAGE:2 * NPAGE, :],
                            1: r_B[:], 2: r_R[:]}
            return acc_of, relus

        def emit_nyr(b, front, acc_of, relus):
            """nyr assembled in PSUM by PE (noise + offload + interp),
            evacuated to fp16 SBUF by ACT, halo cols padded by Pool."""
            _ptiles, noi = front
            nyrtiles = []
            for qi, (r, c) in enumerate(QUADS):
                pa = psum_ac.tile([128, NPAGE, Q], F32, tag="pacc",
                                  name=f"pa{b}_{qi}")
                for pg in range(NPAGE):
                    nc.tensor.matmul(
                        pa[:, pg, :], band16_t[plan["ident16"]],
                        noi[:, r, pg, c:W:2], start=True, stop=False)
                    for k in sorted(relus):
                        bidx = plan["off_bands"][(QUAD_CH[qi], k)]
                        nc.tensor.matmul(
                            pa[:, pg, :], band16_t[bidx],
                            relus[k][qi][:, pg, :], start=False, stop=False)
                    nc.tensor.matmul(
                        pa[:, pg, :], band32_t[plan["ident32"]],
                        acc_of[qi][:, pg, :], start=False, stop=True)
                npr = nyr.tile([128, NPAGE, Q + 2], F16, tag=f"nyr{qi}",
                               name=f"npr{b}_{qi}")
                nc.scalar.copy(out=npr[:, :, 1:Q + 1], in_=pa[:])
                lsrc = 2 if c == 0 else 1
                rsrc = Q if c == 0 else Q - 1
                nc.gpsimd.tensor_copy(out=npr[:, :, 0:1],
                                      in_=npr[:, :, lsrc:lsrc + 1])
                nc.gpsimd.tensor_copy(out=npr[:, :, Q + 1:Q + 2],
                                      in_=npr[:, :, rsrc:rsrc + 1])
                nyrtiles.append(npr)
            return nyrtiles

        def emit_back(b, nyrtiles):
            """raw clips, demosaic + clip/evac, stores."""
            ot = [outp.tile([128, 2, NPAGE, W], F16, tag=f"o{ch}",
                            name=f"ot{b}_{ch}") for ch in range(3)]

            for qi, (r, c) in enumerate(QUADS):
                rch = RAW_OUT[(r, c)]
                for pg in range(NPAGE):
                    eng = engine_of(RAW_CLIP_ENGINE[qi * NPAGE + pg])
                    eng.tensor_scalar(
                        out=ot[rch][:, r, pg, c:W:2],
                        in0=nyrtiles[qi][:, pg, 1:Q + 1],
                        scalar1=0.0, scalar2=1.0,
                        op0=AOT.max, op1=AOT.min)

            ci = 0
            for (r, c), fname, ch, pages in plan["dem"]:
                for page, mains in enumerate(pages):
                    ps = psum_dm.tile([128, Q], F32, tag="dmps",
                                      name=f"dm{b}_{ci}")
                    for i, (bidx, plane, spage, sj) in enumerate(mains):
                        nc.tensor.matmul(
                            ps[:], band16_t[bidx],
                            nyrtiles[plane][:, spage, 1 + sj:1 + sj + Q],
                            start=(i == 0), stop=(i == len(mains) - 1))
                    dst = ot[ch][:, r, page, c:W:2]
                    tcl = cvp.tile([128, Q], F32, tag="conv",
                                   name=f"cv{b}_{ci}")
                    nc.scalar.activation(out=tcl[:], in_=ps[:],
                                         func=ACT_F.Relu)
                    engine_of(DEM_MIN_ENGINE[ci]).tensor_scalar(
                        out=dst, in0=tcl[:], scalar1=1.0, scalar2=None,
                        op0=AOT.min)
                    ci += 1

            # stores from ACT's queue so SP's input loads don't block
            for ch in range(3):
                nc.scalar.dma_start(out=_par_page(out[b, ch]), in_=ot[ch][:])

        # ---- software-pipelined emission ----
        fronts = [emit_front(0), emit_front(1)]
        blur = emit_blur(0, fronts[0])
        for b in range(B_LOC):
            acc_of, relus = emit_chains(b, blur)
            if b + 1 < B_LOC:
                blur = emit_blur(b + 1, fronts[b + 1])
            if b + 2 < B_LOC:
                fronts.append(emit_front(b + 2))
            nyrtiles = emit_nyr(b, fronts[b], acc_of, relus)
            emit_back(b, nyrtiles)

    nc.compile()
    return nc


# ---------------------------------------------------------------------------
# public entry
# ---------------------------------------------------------------------------

_CACHE = {}


def _get_compiled(yp):
    key = np.asarray(yp, np.float32).tobytes()
    if key not in _CACHE:
        plan = build_plan(yp)
        _CACHE[key] = (build_kernel(plan), plan)
    return _CACHE[key]


def build_in_maps(im, yp, noise):
    im = np.ascontiguousarray(np.asarray(im, np.float32))
    noise = np.asarray(noise, np.float32)
    nc, plan = _get_compiled(np.asarray(yp, np.float32))
    noise_s = np.ascontiguousarray(
        (noise * np.float32(1.0 / 255.0)).astype(np.float16))
    in_maps = []
    for k in range(N_CORES):
        sl = slice(k * B_LOC, (k + 1) * B_LOC)
        in_maps.append({
            "im": im[sl],
            "noise": noise_s[sl],
            "bands32": plan["bands32"],
            "bands16": plan["bands16"],
        })
    return nc, in_maps


def kernel(im, yp, noise):
    nc, in_maps = build_in_maps(im, yp, noise)
    res = run_bass_kernel_spmd(nc, in_maps, core_ids=list(range(N_CORES)))
    return np.concatenate(
        [np.asarray(r["out"], np.float16).astype(np.float32)
         for r in res.results], axis=0)


# revision 13
# speedup vs baseline: 1.1007x; 1.0878x over previous
"""Trainium2 Bass kernel for the Camera ISP pipeline (mosaic -> gaussian blur
-> per-channel piecewise-linear calibration -> noise -> Malvar demosaic -> clip).

v3 strategy (per core, pure data parallel over batch: 4 images/core):
- Row-decimated contiguous DMA loads of the 4 bayer source planes (f32r,
  full width); noise and output travel as fp16 (halves their DMA bytes).
- Vertical blur directly in the quad (polyphase) domain: banded matmuls on
  PE with free dim 256 (only the column parity each family needs), ACT
  evacuation, horizontal blur as Pool pair-sum + DVE combine.
- 17-knot np.interp: chained custom DVE instructions (2 piecewise segments
  each); G quads share one chain. The last OFFLOAD_C kinks run as ACT relu
  planes accumulated by PE diagonal bands into PSUM together with the noise
  and the DVE part (identity bands), so the noisy linear image (nyr) is
  assembled in PSUM and evacuated once by ACT.
- Malvar 5x5 demosaic as fp16 banded matmuls accumulating in PSUM.
- Clip/evac: ACT relu (PSUM->SBUF) + Pool min, writing fp16 output tiles;
  3 output DMAs per image issued from ACT's queue so SP's input-load queue
  never blocks behind them.
- Software-pipelined emission: the next image's blur front is emitted
  between this image's interp chains and its demosaic, so PE/ACT/Pool keep
  working while DVE runs the serial chains.
"""

import sys

sys.path.insert(0, "/opt/trn_rl_repo")

import numpy as np

import concourse.bacc as bacc
import concourse.bass as bass
import concourse.tile as tile
from concourse import mybir
from concourse.bass_utils import run_bass_kernel_spmd
from concourse import dve_ops as _dops
from concourse.dve_spec import (
    C0, C1, C2, C3, One, Spec, Src0, Src1,
    _has_src1, _spill_c3_to_src1, lower, relu,
)
from concourse.dve_uop import DveOpSpec

F32 = mybir.dt.float32
F32R = mybir.dt.float32r
F16 = mybir.dt.float16
AOT = mybir.AluOpType
ACT_F = mybir.ActivationFunctionType

B_TOT, H, W = 32, 512, 512
N_CORES = 8
B_LOC = B_TOT // N_CORES           # images per core
Q = H // 2                          # quad-plane dim (256)
NPAGE = Q // 128                    # pages per quad plane (2)
DELTA = 255.0 / 16.0                # knot spacing of the LUT

# ---- tuning knobs ---------------------------------------------------------
OFFLOAD_C = 2          # trailing kinks on ACT+PE instead of DVE (even)
# engine for the second clip op (min) of the 16 demosaic tiles
DEM_MIN_ENGINE = ["gpsimd"] * 16
# engine for each of the 8 raw clip tiles
RAW_CLIP_ENGINE = ["gpsimd"] * 8


# ---------------------------------------------------------------------------
# custom DVE ops (2 LUT segments per instruction)
# ---------------------------------------------------------------------------

def _head_ref(in0, in1, s0, s1, imm2):
    p = in0.shape[0]
    x = np.asarray(in0, np.float32).reshape(p, -1)
    d1 = np.asarray(in1, np.float32).reshape(p, 1)
    return (s0 * x + s1) + d1 * np.maximum(x - 1.0, 0.0)


def _pair_ref(in0, in1, s0, s1, imm2):
    x = np.asarray(in0, np.float32)
    acc = np.asarray(in1, np.float32).reshape(x.shape)
    return (acc + s0 * np.maximum(x - imm2, 0.0)) + s1 * np.maximum(
        x - imm2 - 1.0, 0.0
    )


def _register_op(name, spec):
    for op in _dops.OPS:
        if op.name == name:
            return op
    row = _dops._CUSTOM_DVE_ROW_BASE + len(_dops.OPS)
    assert row < 0x20, "custom DVE opcode rows exhausted"
    _dops._SUB_OPCODE_FOR_NAME[name] = row
    shas = {}
    for ver in ("v3", "v4"):
        try:
            s = DveOpSpec(name=name, opcode=row, uops=lower(spec, ver=ver),
                          rd1_en=_has_src1(spec))
            shas[ver] = s.sha(ver)
        except Exception:
            pass
    op = _dops.DveOp(name, spec, subdim=False, uops_sha=shas)
    _dops.OPS.append(op)
    _dops.CUSTOM_DVE_SPECS[name] = spec
    return op


INTERP_HEAD = _register_op(
    "CAM_INTERP_HEAD",
    Spec(body=_spill_c3_to_src1((C0 * Src0 + C1) + C3 * relu(Src0 - One)),
         reference=_head_ref),
)
INTERP_PAIR = _register_op(
    "CAM_INTERP_PAIR",
    Spec(body=(Src1 + C0 * relu(Src0 - C2)) + C1 * relu(Src0 - (C2 + One)),
         reference=_pair_ref),
)


# ---------------------------------------------------------------------------
# host-side constant planning
# ---------------------------------------------------------------------------

def _gauss1d(sigma=0.4):
    x = np.array([-1.0, 0.0, 1.0], dtype=np.float64)
    g = np.exp(-(x ** 2) / (2.0 * sigma ** 2))
    g /= g.sum()
    return g.astype(np.float32)  # [g0, g1, g0]


_G_AT = np.array([[0, 0, -1, 0, 0], [0, 0, 2, 0, 0], [-1, 2, 4, 2, -1],
                  [0, 0, 2, 0, 0], [0, 0, -1, 0, 0]], np.float32) / 8.0
_K_H = np.array([[0, 0, 0.5, 0, 0], [0, -1, 0, -1, 0], [-1, 4, 5, 4, -1],
                 [0, -1, 0, -1, 0], [0, 0, 0.5, 0, 0]], np.float32) / 8.0
_K_V = _K_H.T.copy()
_K_D = np.array([[0, 0, -1.5, 0, 0], [0, 2, 0, 2, 0], [-1.5, 0, 6, 0, -1.5],
                 [0, 2, 0, 2, 0], [0, 0, -1.5, 0, 0]], np.float32) / 8.0
FILTS = {"G": _G_AT, "H": _K_H, "V": _K_V, "D": _K_D}

CONV_OUT = [
    ((0, 0), "V", 0), ((0, 1), "D", 0),
    ((0, 0), "H", 2), ((0, 1), "G", 1),
    ((1, 0), "G", 1), ((1, 1), "H", 0),
    ((1, 0), "D", 2), ((1, 1), "V", 2),
]
RAW_OUT = {(0, 0): 1, (0, 1): 2, (1, 0): 0, (1, 1): 1}  # quad -> raw channel

# quad planes: 0=G1(even,even) 1=B(even,odd) 2=R(odd,even) 3=G2(odd,odd)
# loaded planes: 0=Gev 1=Rod 2=Bev 3=God  (Gev+Rod first: quad 0's vblur
# can start after the first two loads)
PLANE_SRC = {0: (0, 0), 1: (2, 1), 2: (1, 0), 3: (3, 1)}
LOAD_SRCS = [(1, 0), (0, 1), (2, 0), (1, 1)]  # (im channel, row parity)
QUAD_CH = {0: 1, 1: 2, 2: 0, 3: 1}   # quad idx -> lut channel (G,B,R,G)
QUADS = ((0, 0), (0, 1), (1, 0), (1, 1))


class _BandBuilder:
    def __init__(self):
        self.mats = []
        self._idx = {}

    def add(self, m):
        key = m.tobytes()
        if key not in self._idx:
            self._idx[key] = len(self.mats)
            self.mats.append(m.copy())
        return self._idx[key]


def build_plan(yp):
    """All host-derived constants. yp: [3,17] float32 (255-domain)."""
    yp = np.asarray(yp, np.float32)
    g = _gauss1d()
    g0, g1 = float(g[0]), float(g[1])
    scale_v = 255.0 * g1 / DELTA
    rho = g0 / g1
    cd, co = scale_v * g1, scale_v * g0

    I = np.eye(128, dtype=np.float32)
    sub = np.zeros((128, 128), np.float32)
    for m in range(1, 128):
        sub[m - 1, m] = 1.0
    sup = np.zeros((128, 128), np.float32)
    for m in range(127):
        sup[m + 1, m] = 1.0

    vb32 = _BandBuilder()
    b_cen = vb32.add(cd * I)
    m_up0 = co * (I + sub)
    m_up0[0, 0] += co
    b_up0 = vb32.add(m_up0)
    b_up1 = vb32.add(co * (I + sub))
    m_upx = np.zeros((128, 128), np.float32)
    m_upx[127, 0] = co
    b_upx = vb32.add(m_upx)
    b_dn0 = vb32.add(co * (I + sup))
    m_dnx = np.zeros((128, 128), np.float32)
    m_dnx[0, 127] = co
    b_dnx = vb32.add(m_dnx)
    m_dn1 = co * (I + sup)
    m_dn1[127, 127] += co
    b_dn1 = vb32.add(m_dn1)
    ident32 = vb32.add(I)

    vb_fams = {
        0: (0, 2, "up"), 1: (1, 3, "up"),
        2: (2, 0, "dn"), 3: (3, 1, "dn"),
    }
    vb_mm = {}
    for qi, (cen, nei, d) in vb_fams.items():
        pages = []
        for pg in range(NPAGE):
            mm = [(b_cen, cen, pg)]
            if d == "up":
                if pg == 0:
                    mm.append((b_up0, nei, 0))
                else:
                    mm.append((b_up1, nei, 1))
                    mm.append((b_upx, nei, 0))
            else:
                if pg == 0:
                    mm.append((b_dn0, nei, 0))
                    mm.append((b_dnx, nei, 1))
                else:
                    mm.append((b_dn1, nei, 1))
            pages.append(mm)
        vb_mm[qi] = pages

    bb = _BandBuilder()

    def reflected_src(i_src, pr):
        if 0 <= i_src < Q:
            return i_src
        y_src = 2 * i_src + pr
        y_r = -y_src if y_src < 0 else 2 * (H - 1) - y_src
        assert y_r % 2 == pr
        return y_r // 2

    def emit_groups(groups, page):
        mats = {}
        for (plane, sj), terms in sorted(groups.items()):
            for m in range(128):
                i_out = 128 * page + m
                for si, w, pr in terms:
                    i_src = reflected_src(i_out + si, pr)
                    sp, k = i_src // 128, i_src % 128
                    key = (plane, sp, sj)
                    if key not in mats:
                        mats[key] = np.zeros((128, 128), np.float32)
                    mats[key][k, m] += w
        return [(bb.add(mat), plane, sp, sj)
                for (plane, sp, sj), mat in sorted(mats.items(),
                                                   key=lambda x: x[0])]

    dem = []
    for (r, c), fname, ch in CONV_OUT:
        K = FILTS[fname]
        groups = {}
        for dy in range(-2, 3):
            for dx in range(-2, 3):
                w = float(K[2 + dy, 2 + dx])
                if w == 0.0:
                    continue
                pr = (r + dy) % 2
                si = (r + dy - pr) // 2
                pc = (c + dx) % 2
                sj = (c + dx - pc) // 2
                plane = 2 * pr + pc
                groups.setdefault((plane, sj), []).append((si, w, pr))
        pages = [emit_groups(groups, page) for page in range(NPAGE)]
        dem.append(((r, c), fname, ch, pages))

    yps = yp / 255.0
    interp = []
    for ch in range(3):
        y0 = float(yps[ch, 0])
        s = np.diff(yps[ch]).astype(np.float64)
        d = np.diff(s)
        interp.append({
            "y0": y0, "s0": float(s[0]),
            "d": [float(v) for v in d],
        })

    off_bands = {}
    ident16 = bb.add(I)
    for ch in range(3):
        for k in range(16 - OFFLOAD_C, 16):
            off_bands[(ch, k)] = bb.add(
                np.float32(interp[ch]["d"][k - 1]) * I)

    bands32 = np.stack(vb32.mats).astype(np.float32)
    bands16 = np.stack(bb.mats).astype(np.float16)
    return {
        "bands32": bands32, "bands16": bands16,
        "vb_mm": vb_mm, "dem": dem, "interp": interp, "rho": rho,
        "off_bands": off_bands, "ident16": ident16, "ident32": ident32,
    }


# ---------------------------------------------------------------------------
# kernel builder
# ---------------------------------------------------------------------------

def _row_decimated(ap2d, parity):
    rows = ap2d.rearrange("(r two) w -> two r w", two=2)[parity]
    return rows.rearrange("(q p) w -> p q w", p=128)


def _par_page(ap2d):
    """[512, 512] DRAM AP -> [128, 2(par), NPAGE, 512]."""
    return ap2d.rearrange("(q p t) w -> p t q w", p=128, t=2)


def build_kernel(plan):
    nc = bacc.Bacc(None, target_bir_lowering=False, debug=False)
    im = nc.dram_tensor("im", [B_LOC, 3, H, W], F32R,
                        kind="ExternalInput").ap()
    noise = nc.dram_tensor("noise", [B_LOC, 1, H, W], F16,
                           kind="ExternalInput").ap()
    n32 = plan["bands32"].shape[0]
    n16 = plan["bands16"].shape[0]
    bands32_d = nc.dram_tensor("bands32", [n32, 128, 128], F32R,
                               kind="ExternalInput").ap()
    bands16_d = nc.dram_tensor("bands16", [n16, 128, 128], F16,
                               kind="ExternalInput").ap()
    out = nc.dram_tensor("out", [B_LOC, 3, H, W], F16,
                         kind="ExternalOutput").ap()

    rho = plan["rho"]
    itp = plan["interp"]
    n_kink_dve = 15 - OFFLOAD_C
    assert (n_kink_dve - 1) % 2 == 0, "OFFLOAD_C must be even"
    n_pairs = (n_kink_dve - 1) // 2

    from contextlib import ExitStack
    with tile.TileContext(nc) as tc, ExitStack() as ctx:
        consts = ctx.enter_context(tc.tile_pool(name="consts", bufs=1))
        imp = ctx.enter_context(tc.tile_pool(name="imp", bufs=2))
        nsp = ctx.enter_context(tc.tile_pool(name="nsp", bufs=2))
        sxp = ctx.enter_context(tc.tile_pool(name="sxp", bufs=2))
        xtp = ctx.enter_context(tc.tile_pool(name="xtp", bufs=2))
        accp = ctx.enter_context(tc.tile_pool(name="accp", bufs=2))
        nyr = ctx.enter_context(tc.tile_pool(name="nyr", bufs=2))
        rlp = ctx.enter_context(tc.tile_pool(name="rlp", bufs=2))
        cvp = ctx.enter_context(tc.tile_pool(name="cvp", bufs=4))
        outp = ctx.enter_context(tc.tile_pool(name="outp", bufs=2))
        psum_vb = ctx.enter_context(
            tc.tile_pool(name="psvb", bufs=1, space="PSUM"))
        psum_ac = ctx.enter_context(
            tc.tile_pool(name="psac", bufs=4, space="PSUM"))
        psum_dm = ctx.enter_context(
            tc.tile_pool(name="psdm", bufs=2, space="PSUM"))

        # --- constants (vblur bands first; fp16 dem bands loaded after the
        # first image's planes so compute starts as early as possible) ---
        b32_all = consts.tile([128, n32, 128], F32R, tag="bands32")
        nc.sync.dma_start(out=b32_all,
                          in_=bands32_d.rearrange("n k m -> k n m"))
        band32_t = [b32_all[:, i, :] for i in range(n32)]
        b16_all = consts.tile([128, n16, 128], F16, tag="bands16")

        def load_bands16():
            nc.sync.dma_start(out=b16_all,
                              in_=bands16_d.rearrange("n k m -> k n m"))
        band16_t = [b16_all[:, i, :] for i in range(n16)]
        d1_t = consts.tile([128, 3], F32, tag="d1")
        for ch in range(3):
            nc.vector.memset(d1_t[:, ch:ch + 1], itp[ch]["d"][0])
        kb_t = consts.tile([128, max(OFFLOAD_C, 1)], F32, tag="kbias")
        for i, k in enumerate(range(16 - OFFLOAD_C, 16)):
            nc.vector.memset(kb_t[:, i:i + 1], -float(k))

        def engine_of(name):
            return {"act": nc.scalar, "gpsimd": nc.gpsimd,
                    "vector": nc.vector}[name]

        def emit_front(b):
            ptiles = []
            for pi, (ch, par) in enumerate(LOAD_SRCS):
                t = imp.tile([128, NPAGE, W], F32R, tag=f"plane{pi}",
                             name=f"pl{b}_{pi}")
                nc.sync.dma_start(out=t, in_=_row_decimated(im[b, ch], par))
                ptiles.append(t)
            noi = nsp.tile([128, 2, NPAGE, W], F16, tag="noise",
                           name=f"ns{b}")
            nc.sync.dma_start(out=noi, in_=_par_page(noise[b, 0]))
            return ptiles, noi

        def emit_blur(b, front):
            """vblur (PE) -> evac (ACT) -> hblur (Pool pair + DVE stt)."""
            ptiles, _noi = front
            vbt = {}
            for qi in range(4):
                ps = psum_vb.tile([128, NPAGE, Q], F32,
                                  tag=f"vb{'AB'[qi % 2]}",
                                  name=f"vps{b}_{qi}")
                for pg, mm in enumerate(plan["vb_mm"][qi]):
                    for i, (bidx, plane, spage) in enumerate(mm):
                        src_pi, cpar = PLANE_SRC[plane]
                        nc.tensor.matmul(
                            ps[:, pg, :], band32_t[bidx],
                            ptiles[src_pi][:, spage, cpar:W:2],
                            start=(i == 0), stop=(i == len(mm) - 1))
                vs = sxp.tile([128, NPAGE, Q], F32, tag=f"vbs{qi}",
                              name=f"vbs{b}_{qi}")
                nc.scalar.copy(out=vs[:], in_=ps[:])
                vbt[qi] = vs

            x_G = xtp.tile([128, 2 * NPAGE, Q], F32, tag="xG", name=f"xG{b}")
            x_B = xtp.tile([128, NPAGE, Q], F32, tag="xB", name=f"xB{b}")
            x_R = xtp.tile([128, NPAGE, Q], F32, tag="xR", name=f"xR{b}")
            x_of = {0: x_G[:, 0:NPAGE, :], 3: x_G[:, NPAGE:2 * NPAGE, :],
                    1: x_B[:], 2: x_R[:]}
            for qi, (r, c) in enumerate(QUADS):
                cen = vbt[qi]
                nei = vbt[2 * r + (1 - c)]
                s = sxp.tile([128, NPAGE, Q], F32, tag="s", name=f"s{b}_{qi}")
                if c == 0:
                    nc.gpsimd.tensor_add(out=s[:, :, 1:Q],
                                         in0=nei[:, :, 0:Q - 1],
                                         in1=nei[:, :, 1:Q])
                    nc.gpsimd.tensor_scalar_mul(
                        out=s[:, :, 0:1], in0=nei[:, :, 0:1], scalar1=2.0)
                else:
                    nc.gpsimd.tensor_add(out=s[:, :, 0:Q - 1],
                                         in0=nei[:, :, 0:Q - 1],
                                         in1=nei[:, :, 1:Q])
                    nc.gpsimd.tensor_scalar_mul(
                        out=s[:, :, Q - 1:Q], in0=nei[:, :, Q - 1:Q],
                        scalar1=2.0)
                nc.vector.scalar_tensor_tensor(
                    out=x_of[qi], in0=s, scalar=rho, in1=cen[:],
                    op0=AOT.mult, op1=AOT.add)
            return {"x_G": x_G, "x_B": x_B, "x_R": x_R, "x_of": x_of}

        def emit_chains(b, blur):
            """Custom-DVE interp chains + ACT relu planes for offload."""
            def interp_chain(ch, xt, shape_free, tagc):
                co = itp[ch]
                xf = xt.rearrange("p a b -> p (a b)")
                a0 = accp.tile([128, shape_free, Q], F32R, tag=f"a0{tagc}",
                               name=f"a0{b}{tagc}")
                a1 = accp.tile([128, shape_free, Q], F32R, tag=f"a1{tagc}",
                               name=f"a1{b}{tagc}")
                nc.vector._custom_dve(
                    INTERP_HEAD, out=a0[:].rearrange("p a b -> p (a b)"),
                    in0=xf, in1=d1_t[:, ch:ch + 1],
                    s0=co["s0"], s1=co["y0"])
                src, dst = a0, a1
                for j in range(1, n_pairs + 1):
                    nc.vector._custom_dve(
                        INTERP_PAIR,
                        out=dst[:].rearrange("p a b -> p (a b)"),
                        in0=xf,
                        in1=src[:].rearrange("p a b -> p (a b)"),
                        s0=co["d"][2 * j - 1], s1=co["d"][2 * j],
                        imm2=float(2 * j))
                    src, dst = dst, src
                return src

            acc_G = interp_chain(1, blur["x_G"][:], 2 * NPAGE, "G")
            acc_B = interp_chain(2, blur["x_B"][:], NPAGE, "B")
            acc_R = interp_chain(0, blur["x_R"][:], NPAGE, "R")
            acc_of = {0: acc_G[:, 0:NPAGE, :],
                      3: acc_G[:, NPAGE:2 * NPAGE, :],
                      1: acc_B[:], 2: acc_R[:]}

            relus = {}
            for k in range(16 - OFFLOAD_C, 16):
                r_G = rlp.tile([128, 2 * NPAGE, Q], F16, tag="rG",
                               name=f"rG{b}_{k}")
                r_B = rlp.tile([128, NPAGE, Q], F16, tag="rB",
                               name=f"rB{b}_{k}")
                r_R = rlp.tile([128, NPAGE, Q], F16, tag="rR",
                               name=f"rR{b}_{k}")
                kb = kb_t[:, k - (16 - OFFLOAD_C):k - (16 - OFFLOAD_C) + 1]
                nc.scalar.activation(out=r_G[:], in_=blur["x_G"][:],
                                     func=ACT_F.Relu, bias=kb)
                nc.scalar.activation(out=r_B[:], in_=blur["x_B"][:],
                                     func=ACT_F.Relu, bias=kb)
                nc.scalar.activation(out=r_R[:], in_=blur["x_R"][:],
                                     func=ACT_F.Relu, bias=kb)
                relus[k] = {0: r_G[:, 0:NPAGE, :],
                            3: r_G[:, NPAGE:2 * NPAGE, :],
                            1: r_B[:], 2: r_R[:]}
            return acc_of, relus

        def emit_nyr(b, front, acc_of, relus):
            """nyr assembled in PSUM by PE (noise + offload + interp),
            evacuated to fp16 SBUF by ACT, halo cols padded by Pool."""
            _ptiles, noi = front
            nyrtiles = []
            for qi, (r, c) in enumerate(QUADS):
                pa = psum_ac.tile([128, NPAGE, Q], F32, tag="pacc",
                                  name=f"pa{b}_{qi}")
                for pg in range(NPAGE):
                    nc.tensor.matmul(
                        pa[:, pg, :], band16_t[plan["ident16"]],
                        noi[:, r, pg, c:W:2], start=True, stop=False)
                    for k in sorted(relus):
                        bidx = plan["off_bands"][(QUAD_CH[qi], k)]
                        nc.tensor.matmul(
                            pa[:, pg, :], band16_t[bidx],
                            relus[k][qi][:, pg, :], start=False, stop=False)
                    nc.tensor.matmul(
                        pa[:, pg, :], band32_t[plan["ident32"]],
                        acc_of[qi][:, pg, :], start=False, stop=True)
                npr = nyr.tile([128, NPAGE, Q + 2], F16, tag=f"nyr{qi}",
                               name=f"npr{b}_{qi}")
                nc.scalar.copy(out=npr[:, :, 1:Q + 1], in_=pa[:])
                lsrc = 2 if c == 0 else 1
                rsrc = Q if c == 0 else Q - 1
                nc.gpsimd.tensor_copy(out=npr[:, :, 0:1],
                                      in_=npr[:, :, lsrc:lsrc + 1])
                nc.gpsimd.tensor_copy(out=npr[:, :, Q + 1:Q + 2],
                                      in_=npr[:, :, rsrc:rsrc + 1])
                nyrtiles.append(npr)
            return nyrtiles

        def emit_back(b, nyrtiles):
            """raw clips, demosaic + clip/evac, stores."""
            ot = [outp.tile([128, 2, NPAGE, W], F16, tag=f"o{ch}",
                            name=f"ot{b}_{ch}") for ch in range(3)]

            for qi, (r, c) in enumerate(QUADS):
                rch = RAW_OUT[(r, c)]
                for pg in range(NPAGE):
                    eng = engine_of(RAW_CLIP_ENGINE[qi * NPAGE + pg])
                    eng.tensor_scalar(
                        out=ot[rch][:, r, pg, c:W:2],
                        in0=nyrtiles[qi][:, pg, 1:Q + 1],
                        scalar1=0.0, scalar2=1.0,
                        op0=AOT.max, op1=AOT.min)

            ci = 0
            for (r, c), fname, ch, pages in plan["dem"]:
                for page, mains in enumerate(pages):
                    ps = psum_dm.tile([128, Q], F32, tag="dmps",
                                      name=f"dm{b}_{ci}")
                    for i, (bidx, plane, spage, sj) in enumerate(mains):
                        nc.tensor.matmul(
                            ps[:], band16_t[bidx],
                            nyrtiles[plane][:, spage, 1 + sj:1 + sj + Q],
                            start=(i == 0), stop=(i == len(mains) - 1))
                    dst = ot[ch][:, r, page, c:W:2]
                    tcl = cvp.tile([128, Q], F32, tag="conv",
                                   name=f"cv{b}_{ci}")
                    nc.scalar.activation(out=tcl[:], in_=ps[:],
                                         func=ACT_F.Relu)
                    engine_of(DEM_MIN_ENGINE[ci]).tensor_scalar(
                        out=dst, in0=tcl[:], scalar1=1.0, scalar2=None,
                        op0=AOT.min)
                    ci += 1

            # stores from ACT's queue so SP's input loads don't block
            for ch in range(3):
                nc.scalar.dma_start(out=_par_page(out[b, ch]), in_=ot[ch][:])

        # ---- software-pipelined emission ----
        fronts = [emit_front(0)]
        load_bands16()
        fronts.append(emit_front(1))
        blur = emit_blur(0, fronts[0])
        for b in range(B_LOC):
            acc_of, relus = emit_chains(b, blur)
            if b + 1 < B_LOC:
                blur = emit_blur(b + 1, fronts[b + 1])
            if b + 2 < B_LOC:
                fronts.append(emit_front(b + 2))
            nyrtiles = emit_nyr(b, fronts[b], acc_of, relus)
            emit_back(b, nyrtiles)

    nc.compile()
    return nc


# ---------------------------------------------------------------------------
# public entry
# ---------------------------------------------------------------------------

_CACHE = {}


def _get_compiled(yp):
    key = np.asarray(yp, np.float32).tobytes()
    if key not in _CACHE:
        plan = build_plan(yp)
        _CACHE[key] = (build_kernel(plan), plan)
    return _CACHE[key]


def build_in_maps(im, yp, noise):
    im = np.ascontiguousarray(np.asarray(im, np.float32))
    noise = np.asarray(noise, np.float32)
    nc, plan = _get_compiled(np.asarray(yp, np.float32))
    noise_s = np.ascontiguousarray(
        (noise * np.float32(1.0 / 255.0)).astype(np.float16))
    in_maps = []
    for k in range(N_CORES):
        sl = slice(k * B_LOC, (k + 1) * B_LOC)
        in_maps.append({
            "im": im[sl],
            "noise": noise_s[sl],
            "bands32": plan["bands32"],
            "bands16": plan["bands16"],
        })
    return nc, in_maps


def kernel(im, yp, noise):
    nc, in_maps = build_in_maps(im, yp, noise)
    res = run_bass_kernel_spmd(nc, in_maps, core_ids=list(range(N_CORES)))
    return np.concatenate(
        [np.asarray(r["out"], np.float16).astype(np.float32)
         for r in res.results], axis=0)
